# revision 42
# baseline (speedup 1.0000x reference)
"""Trainium2 Bass kernel for ConvBnSign (binarized 3x3 conv + sync-BN + sign).

Math: y = conv2d(x, sign(w) * alpha)  with alpha = mean|w| per out-channel,
then train-mode BatchNorm over (N,H,W), then hard_sign.

Folds: alpha > 0 folds into the BN affine; S = sign(gamma) folds into the
binarized weights (z' = S*z), making the BN scale A = alpha*|gamma|*rsqrt(
alpha^2 var + eps) >= 0 so the final sign is also a per-channel threshold
compare  out = (z' >= T) ? +1 : -1,  T = mu' - beta/A  — which lets the
tail chunk's sign pass run on ACT (Sign activation) and DVE (is_ge
compare emitting {1,0}, remapped to +-1 on the host) in parallel.

Precision: x is split on host into 3 fp8-e4m3 terms
  x ~ q0 + q1/16 + q2/64   (residual rms ~1.6e-5 relative),
with per-term scales folded into the fp8 weights (1, 2^-4, 2^-6 — all
normal in e4m3). Each conv tile is 14 DoubleRow fp8 matmuls (2 k-tiles
of 128, 0.5 cycles/row — 4x bf16 contraction rate): the 27 (tap, term)
k-tiles pair via the plane dim of the SBUF layout, where two extra
planes are host-shifted copies of q2 (shift +1 / +59) so cross-tap pairs
land at the uniform plane stride.

Schedule: dummy matmuls pre-ramp the PE clock while the first DMAs land
(3-piece img0 transfer so conv starts ~4.7 us); per finished PSUM tile,
DVE copies z (+sum accum) and ACT squares (+sumsq accum); chunk 0's
BN/sign work is emitted inside chunk 1's conv stream (2 sign halves per
conv window, within ACT's slack); the tail chunk's signs split
ACT/DVE with piece sizes balanced to each engine's rate, staged in
half-image fp8 tiles whose DMAs are interleaved by expected completion
(DVE pieces via Pool-issued SWDGE DMAs to bypass the serialized HWDGE).

Sharding: data-parallel, 4 images per core across 8 cores; BN stats are
per-channel partial sums [128,2] fp32 all-reduced across cores.
"""

import numpy as np
import ml_dtypes

import concourse.bass as bass
import concourse.mybir as mybir
import concourse.tile as tile
from concourse.vector_clock import ScopedClock
from concourse.bass_utils import run_bass_kernel_spmd

# ---- problem constants (hardcoded per contract) ----
N_CORES = 8
N_FULL = 32           # batch
CIN = 128             # input channels
COUT = 256            # output channels
H = W = 56
KH = KW = 3
BN_EPS = 1e-5

IMGS = N_FULL // N_CORES          # 4 images per core
WP = W + 2                        # 58 padded width
HP = H + 2
PADPIX = HP * WP                  # 3364
PIX = H * W                       # 3136
NCHUNK = COUT // 128              # 2 chunks of 128 output channels
RTR = 8                           # rows per matmul tile
RT = H // RTR                     # 7 row tiles per image
NTILE = RTR * W                   # 448 = matmul free dim (<=512, one PSUM bank)
NTOT = N_FULL * PIX               # 200704 elements per channel for BN stats
NPAIR = 14                        # DoubleRow matmuls per PSUM tile
HFX = PIX // 2                    # half-image columns (sign/DMA granularity)
WSLOT = 2 * 128                   # fp8 bytes per weight pair
PHA = 9 * WSLOT                   # phase-1 weight bytes (pairs 0-8)
PHB = 5 * WSLOT                   # phase-2 weight bytes (pairs 9-13)

BF16 = mybir.dt.bfloat16
F32 = mybir.dt.float32
FP8 = mybir.dt.float8e4
NP8 = ml_dtypes.float8_e4m3

_MAX_DRAIN_WAITS = 1  # walrus CTRL instructions accept a single sync wait


def _split_multi_waits(nc, max_waits=1):
    """This walrus build rejects instructions with more than one sem wait.
    Hoist excess waits onto same-engine NoOps inserted immediately before the
    offending instruction (the engine blocks at the NoOp instead — identical
    ordering semantics)."""
    ctr = 0
    for bbw in nc.main_func.blocks:
        out = []
        changed = False
        for inst in bbw.instructions:
            si = inst.sync_info
            w = list(si.on_wait or []) if si else []
            if len(w) > max_waits:
                changed = True
                excess = w[: len(w) - max_waits]
                for i in range(0, len(excess), max_waits):
                    nop = mybir.InstNoOp(name=f"WFIX-{ctr}", ins=[], outs=[])
                    ctr += 1
                    nop.engine = inst.engine
                    nop.sync_info = mybir.SyncInfo(
                        on_wait=excess[i : i + max_waits], on_update=[]
                    )
                    out.append(nop)
                inst.sync_info = mybir.SyncInfo(
                    on_wait=w[len(w) - max_waits :],
                    on_update=list(si.on_update or []),
                )
            out.append(inst)
        if changed:
            bbw.instructions = out
    return ctr


class _SplitDrainTileContext(tile.TileContext):
    """TileContext whose final drain splits its sem waits across multiple
    sync-engine instructions (this walrus build caps CTRL waits at 1)."""

    def _drain_and_barrier(self, tick_clock, wait_clock):
        drain_inst = self.nc.sync.drain()
        wait_clock.add_sem_waits(
            drain_inst.ins, ScopedClock({None: tick_clock.global_clock})
        )
        si = drain_inst.ins.sync_info
        w = list(si.on_wait or [])
        if len(w) > _MAX_DRAIN_WAITS:
            drain_inst.ins.sync_info = mybir.SyncInfo(
                on_wait=w[:_MAX_DRAIN_WAITS], on_update=list(si.on_update or [])
            )
            for i in range(_MAX_DRAIN_WAITS, len(w), _MAX_DRAIN_WAITS):
                nop = self.nc.sync.nop(nofuse=True)
                nop.ins.sync_info = mybir.SyncInfo(
                    on_wait=w[i : i + _MAX_DRAIN_WAITS], on_update=[]
                )
        self.nc.all_engine_barrier()
        assert self.sems is not None
        popped = self.nc._tile_sem_poison_stack.pop()
        assert popped is self._sem_poison
        self.nc.clear_and_free_semaphores(list(self.sems.allocated().values()))
        self.nc.all_engine_barrier()


def _pair_rhs(xa_v, xb_v, r0, nr, pr):
    """rhs AP [128, 2, nr, 56] for DoubleRow pair pr of the row-tile
    covering output rows [r0, r0+nr).

    xa_v: [128, 2, HP, WP] view of planes (q0, q1);
    xb_v: [128, 3, HP, WP] view of planes (q2, q2<<1, q2<<59)."""
    if pr < 9:                       # (tap pr, q0) + (tap pr, q1)
        dy, dx = divmod(pr, KW)
        return xa_v[:, 0:2, r0 + dy : r0 + dy + nr, dx : dx + W]
    if pr < 12:                      # q2 taps (dy,0)+(dy,1) via shift-1 plane
        dy = pr - 9
        return xb_v[:, 0:2, r0 + dy : r0 + dy + nr, 0:W]
    if pr == 12:                     # q2 taps (0,2)+(1,2) via shift-59 plane
        return xb_v[:, 1:3, r0 : r0 + nr, 1 : 1 + W]
    # pr == 13: q2 tap (2,2), second half has zero weights
    return xb_v[:, 0:1, r0 + 2 : r0 + 2 + nr, 2 : 2 + W].broadcast_to(
        [128, 2, nr, W]
    )


def build_bass(n_cores=N_CORES, collective=True):
    """Build the per-core Bass module (SPMD: same program on every core)."""
    nc = bass.Bass(num_devices=n_cores)

    xq_d = nc.dram_tensor("xq", [IMGS, CIN, 5 * PADPIX], FP8,
                          kind="ExternalInput")
    wq_d = nc.dram_tensor("wq", [CIN, NCHUNK * NPAIR * WSLOT], FP8,
                          kind="ExternalInput")
    # pqrb[p, j-col chunks]: P | Qc | R | beta  (4 cols per chunk)
    pqrb_d = nc.dram_tensor("pqrb", [128, 4 * NCHUNK], F32,
                            kind="ExternalInput")
    out_d = nc.dram_tensor("out", [IMGS, NCHUNK, 128, PIX], FP8,
                           kind="ExternalOutput")

    with _SplitDrainTileContext(nc) as tc:
        with (
            tc.tile_pool(name="const", bufs=1) as constp,
            tc.tile_pool(name="xbuf", bufs=1) as xp,
            tc.tile_pool(name="zbuf", bufs=1) as zp,
            tc.tile_pool(name="stats", bufs=1) as sp,
            tc.tile_pool(name="sq", bufs=2) as sqp,
            tc.tile_pool(name="ost", bufs=6) as op_,
            tc.tile_pool(name="pz", bufs=8, space="PSUM") as pp,
            tc.tile_pool(name="dram", bufs=1, space="DRAM") as dp,
        ):
            # ---- weights (split per chunk/phase for early PE start) ----
            wa = [constp.tile([128, PHA], FP8, tag=f"wa{j}", name=f"wa{j}")
                  for j in range(NCHUNK)]
            wb = [constp.tile([128, PHB], FP8, tag=f"wb{j}", name=f"wb{j}")
                  for j in range(NCHUNK)]
            pqrb_sb = constp.tile([128, 4 * NCHUNK], F32, tag="pqrb")

            # ---- x plane stacks (per image, split planes 01 / 234) ----
            xa = [xp.tile([128, 2 * PADPIX], FP8, tag=f"xa{i}", name=f"xa{i}")
                  for i in range(IMGS)]
            xb = [xp.tile([128, 3 * PADPIX], FP8, tag=f"xb{i}", name=f"xb{i}")
                  for i in range(IMGS)]

            # DMA issue order = arrival order: w0a, img0 planes, w0b, then
            # the rest.  First matmul waits only on w0a + xa[0].
            xa0_v = xa[0][:].rearrange("p (t pix) -> p t pix", t=2)
            xq0_v = xq_d[0].rearrange("c (t pix) -> c t pix", t=5)[:, 0:2]
            R1, R2 = 12 * WP, 30 * WP
            nc.sync.dma_start(xa0_v[:, :, 0:R1], xq0_v[:, :, 0:R1])
            nc.sync.dma_start(wa[0][:], wq_d[:, 0:PHA])
            nc.sync.dma_start(xa0_v[:, :, R1:R2], xq0_v[:, :, R1:R2])
            nc.sync.dma_start(xa0_v[:, :, R2:], xq0_v[:, :, R2:])
            nc.sync.dma_start(xb[0][:], xq_d[0][:, 2 * PADPIX :])
            nc.sync.dma_start(wb[0][:], wq_d[:, PHA : PHA + PHB])
            nc.sync.dma_start(pqrb_sb[:], pqrb_d[:])
            for i in range(1, IMGS):
                nc.sync.dma_start(xa[i][:], xq_d[i][:, 0 : 2 * PADPIX])
                nc.sync.dma_start(xb[i][:], xq_d[i][:, 2 * PADPIX :])
            off = NPAIR * WSLOT
            nc.sync.dma_start(wa[1][:], wq_d[:, off : off + PHA])
            nc.sync.dma_start(wb[1][:], wq_d[:, off + PHA : off + PHA + PHB])

            # ---- PE warmup: dummy matmuls ramp the tensor-engine clock
            # to full speed while the x/w DMAs are in flight ----
            wu = constp.tile([128, 8], FP8, tag="warm")
            wscr = constp.tile([128, 1], F32, tag="wscr")
            nc.gpsimd.memset(wu[:], 0)
            pw = pp.tile([128, 512], F32, tag="pz", name="warm_ps")
            for i in range(7):
                nc.tensor.matmul(pw[0:1, :], wu[:, i : i + 1],
                                 wu[:, 0:1].broadcast_to([128, 512]),
                                 start=True, stop=True)
            nc.vector.tensor_copy(wscr[0:1, 0:1], pw[0:1, 0:1])

            # ---- z buffers + stats ----
            z0 = zp.tile([128, IMGS * PIX], F32, tag="z0", name="z0")
            z1 = zp.tile([128, IMGS * PIX], F32, tag="z1", name="z1")
            ssum = sp.tile([128, 64], F32, tag="ssum")

            ssq = sp.tile([128, 64], F32, tag="ssq")

            P_ = pqrb_sb[:, 0:NCHUNK]
            Qc = pqrb_sb[:, NCHUNK : 2 * NCHUNK]
            R_ = pqrb_sb[:, 2 * NCHUNK : 3 * NCHUNK]
            beta = pqrb_sb[:, 3 * NCHUNK : 4 * NCHUNK]
            inv_n = 1.0 / NTOT
            npart = IMGS * RT

            def _zslice(j, img, px0, npx):
                zt = z0 if j == 0 else z1
                return zt[:, img * PIX + px0 : img * PIX + px0 + npx]

            def _consume(j, img, px0, npx, col, pt, sq_dve=False):
                """Stats consumers for one finished PSUM tile: DVE z-copy
                with sum accumulation, square with sumsq accumulation on ACT
                (or on DVE for the very last tile, where ACT's queue wait
                would delay the stats -> BN -> sign chain)."""
                zs = _zslice(j, img, px0, npx)
                nc.vector.tensor_scalar(
                    out=zs, in0=pt[:, 0:npx], scalar1=0.0, scalar2=None,
                    op0=mybir.AluOpType.add, op1=mybir.AluOpType.add,
                    accum_out=ssum[:, col : col + 1],
                )
                sqt = sqp.tile([128, NTILE], F32, tag="sqt")
                if sq_dve:
                    nc.vector.scalar_tensor_tensor(
                        out=sqt[:, 0:npx], in0=pt[:, 0:npx], scalar=1.0,
                        in1=zs, op0=mybir.AluOpType.mult,
                        op1=mybir.AluOpType.mult,
                        accum_out=ssq[:, col : col + 1],
                    )
                else:
                    nc.scalar.activation(
                        out=sqt[:, 0:npx], in_=pt[:, 0:npx],
                        func=mybir.ActivationFunctionType.Square,
                        accum_out=ssq[:, col : col + 1],
                    )

            def _mm(pt, wv, xa_v, xb_v, r0, nr, pr):
                nc.tensor.matmul(
                    pt[:, 0 : nr * W], wv[:, pr] if pr < 9 else wv[:, pr - 9],
                    _pair_rhs(xa_v, xb_v, r0, nr, pr),
                    start=(pr == 0), stop=(pr == 12),
                    perf_mode=mybir.MatmulPerfMode.DoubleRow,
                )

            TILE_ORDER = list(range(9)) + [9, 10, 11, 13, 12]

            def _conv_img(j, img):
                wa_v = wa[j][:].rearrange("p (pr k o) -> p pr k o", pr=9, k=2)
                wb_v = wb[j][:].rearrange("p (pr k o) -> p pr k o", pr=5, k=2)
                xa_v = xa[img][:].rearrange("p (t r c) -> p t r c",
                                            t=2, r=HP)
                xb_v = xb[img][:].rearrange("p (t r c) -> p t r c",
                                            t=3, r=HP)
                tiles = [(rt * RTR, RTR) for rt in range(RT)]
                base = 28 * j + 7 * img
                pts = [pp.tile([128, NTILE], F32, tag="pz",
                               name=f"pz{j}_{img}_{t}")
                       for t in range(len(tiles))]
                if j == 0 and img == 0:
                    # Phased: pairs 0-8 (planes q0/q1, early DMA) across
                    # all row tiles first — a 63-matmul runway while
                    # xb[0] is still in flight.
                    for t, (r0, nr) in enumerate(tiles):
                        for pr in range(9):
                            _mm(pts[t], wa_v, xa_v, xb_v, r0, nr, pr)
                    for t, (r0, nr) in enumerate(tiles):
                        for pr in (9, 10, 11, 13, 12):
                            _mm(pts[t], wb_v, xa_v, xb_v, r0, nr, pr)
                        _consume(j, img, r0 * W, nr * W, base + t, pts[t])
                else:
                    # Tile-major: each tile's 14 pairs are consecutive so
                    # group-closures are spaced a full tile apart and the
                    # per-tile consumers keep up.
                    for t, (r0, nr) in enumerate(tiles):
                        for pr in TILE_ORDER:
                            _mm(pts[t], wa_v if pr < 9 else wb_v,
                                xa_v, xb_v, r0, nr, pr)
                        _consume(j, img, r0 * W, nr * W, base + t, pts[t])

            def _stats_bn(j):
                # ---- chunk-j stats: [128,2] = (sum, sumsq) ----
                ncol = npart
                cc_sb = sp.tile([128, 2], F32, tag=f"ccsb{j}", name=f"ccsb{j}")
                nc.vector.reduce_sum(
                    out=cc_sb[:, 0:1],
                    in_=ssum[:, j * npart : j * npart + ncol],
                    axis=mybir.AxisListType.X,
                )
                nc.vector.reduce_sum(
                    out=cc_sb[:, 1:2],
                    in_=ssq[:, j * npart : j * npart + ncol],
                    axis=mybir.AxisListType.X,
                )
                st = sp.tile([128, 2], F32, tag=f"st{j}", name=f"st{j}")
                if collective and n_cores > 1:
                    cc_in = dp.tile([128, 2], F32, tag=f"ccin{j}",
                                    name=f"ccin{j}")
                    cc_out = dp.tile([128, 2], F32, tag=f"ccout{j}",
                                     name=f"ccout{j}")
                    nc.sync.dma_start(cc_in[:], cc_sb[:])
                    nc.gpsimd.collective_compute(
                        "AllReduce", mybir.AluOpType.add,
                        replica_groups=[list(range(n_cores))],
                        ins=[cc_in.opt()], outs=[cc_out.opt()],
                    )
                    nc.sync.dma_start(st[:], cc_out[:])
                else:
                    st = cc_sb

                # ---- BN affine:  A = R*rsqrt(P*q - Qc*s^2 + eps) >= 0,
                #      B = beta - mu*A,  T = mu - beta/A  (tail chunk only).
                # Chunk 0's math runs on Pool (DVE is busy with tile
                # consumers of chunk 1); the tail chunk's runs on DVE.
                last = j == NCHUNK - 1
                eng = nc.vector if last else nc.gpsimd
                Pj, Qj, Rj, bj = (v[:, j : j + 1] for v in (P_, Qc, R_, beta))
                s0, s1 = st[:, 0:1], st[:, 1:2]
                mu = sp.tile([128, 1], F32, tag=f"mu{j}", name=f"mu{j}")
                u = sp.tile([128, 1], F32, tag=f"u{j}", name=f"u{j}")
                A = sp.tile([128, 1], F32, tag=f"A{j}", name=f"A{j}")
                B = sp.tile([128, 1], F32, tag=f"B{j}", name=f"B{j}")
                t1 = sp.tile([128, 1], F32, tag=f"t1{j}", name=f"t1{j}")

                if last:
                    # fused via scalar_tensor_tensor (DVE-only instruction)
                    nc.vector.scalar_tensor_tensor(
                        out=t1[:], in0=s0, scalar=Qj, in1=s0,
                        op0=mybir.AluOpType.mult, op1=mybir.AluOpType.mult)
                    nc.vector.scalar_tensor_tensor(
                        out=u[:], in0=s1, scalar=Pj, in1=t1[:],
                        op0=mybir.AluOpType.mult,
                        op1=mybir.AluOpType.subtract)
                else:
                    eng.tensor_tensor(out=u[:], in0=s1, in1=Pj,
                                      op=mybir.AluOpType.mult)
                    eng.tensor_tensor(out=t1[:], in0=s0, in1=s0,
                                      op=mybir.AluOpType.mult)
                    eng.tensor_tensor(out=t1[:], in0=t1[:], in1=Qj,
                                      op=mybir.AluOpType.mult)
                    eng.tensor_tensor(out=u[:], in0=u[:], in1=t1[:],
                                      op=mybir.AluOpType.subtract)
                eng.tensor_scalar(out=u[:], in0=u[:], scalar1=float(BN_EPS),
                                  scalar2=None, op0=mybir.AluOpType.add)
                nc.vector.reciprocal(u[:], u[:])
                nc.scalar.activation(out=u[:], in_=u[:],
                                     func=mybir.ActivationFunctionType.Sqrt)
                eng.tensor_tensor(out=A[:], in0=Rj, in1=u[:],
                                  op=mybir.AluOpType.mult)
                eng.tensor_scalar(out=mu[:], in0=s0, scalar1=inv_n,
                                  scalar2=None, op0=mybir.AluOpType.mult)
                eng.tensor_tensor(out=t1[:], in0=mu[:], in1=A[:],
                                  op=mybir.AluOpType.mult)
                eng.tensor_tensor(out=B[:], in0=bj, in1=t1[:],
                                  op=mybir.AluOpType.subtract)
                return A, B, mu, t1, bj

            def _act_sign(j, AB, img, h):
                """ACT Sign on half-image h of img -> staged -> DRAM."""
                A, B = AB
                o = op_.tile([128, HFX], FP8, tag="ostg",
                             name=f"ostg{j}_{img}_{h}")
                nc.scalar.activation(
                    out=o[:], in_=_zslice(j, img, h * HFX, HFX),
                    func=mybir.ActivationFunctionType.Sign,
                    bias=B[:, 0:1], scale=A[:, 0:1],
                )
                nc.sync.dma_start(
                    out_d[img, j][:, h * HFX : (h + 1) * HFX], o[:])

            # ---- emission schedule: chunk 0's stats/BN/signs are emitted
            # inside chunk 1's conv stream so its ACT sign work interleaves
            # with chunk 1's Squares without head-of-line blocking. ----
            for img in range(IMGS):
                _conv_img(0, img)
            _conv_img(1, 0)
            A0, B0, _, _, _ = _stats_bn(0)
            # 2 chunk-0 sign halves per remaining conv window (ACT slack is
            # ~3.95 us/window; 3 halves would spill into the next window)
            _act_sign(0, (A0, B0), 0, 0)
            _act_sign(0, (A0, B0), 0, 1)
            sched0 = [[(1, 0), (1, 1)],
                      [(2, 0), (2, 1)],
                      [(3, 0), (3, 1)]]
            for img in range(1, IMGS):
                _conv_img(1, img)
                for (si, sh) in sched0[img - 1]:
                    _act_sign(0, (A0, B0), si, sh)

            # ---- tail chunk: ACT takes imgs 0-1 + last 392 cols, DVE the
            # rest as single-pass compares (z >= T) -> {1,0}; the host maps
            # {1,0} -> {+1,-1} for exactly those regions. ----
            A1, B1, mu1, t1_, bj1 = _stats_bn(1)
            T = sp.tile([128, 1], F32, tag="T", name="T")
            nc.vector.reciprocal(t1_[:], A1[:])
            nc.vector.tensor_tensor(out=t1_[:], in0=bj1, in1=t1_[:],
                                    op=mybir.AluOpType.mult)
            nc.vector.tensor_tensor(out=T[:], in0=mu1[:], in1=t1_[:],
                                    op=mybir.AluOpType.subtract)

            def _dve_cmp(img, h):
                o = op_.tile([128, HFX], FP8, tag="ostg",
                             name=f"ostg1_{img}_{h}")
                nc.vector.tensor_tensor(
                    out=o[:], in0=_zslice(1, img, h * HFX, HFX),
                    in1=T[:, 0:1].broadcast_to([128, HFX]),
                    op=mybir.AluOpType.is_ge,
                )
                # Pool-issued DMA: SWDGE path, skips the serialized HWDGE
                nc.gpsimd.dma_start(
                    out_d[img, 1][:, h * HFX : (h + 1) * HFX], o[:])

            # interleaved by expected completion so the in-order SP DMA
            # queue never head-of-line blocks on a not-yet-ready piece
            _act_sign(1, (A1, B1), 0, 0)
            _dve_cmp(2, 0)
            _act_sign(1, (A1, B1), 0, 1)
            _dve_cmp(2, 1)
            _act_sign(1, (A1, B1), 1, 0)
            _dve_cmp(3, 0)
            _act_sign(1, (A1, B1), 1, 1)
            # img3 h1: DVE covers cols 0-1176, ACT the last 392; one staging
            # tile, one SP DMA after both writers
            o3 = op_.tile([128, HFX], FP8, tag="ostg", name="ostg1_3_1")
            lo = 3 * PIX + HFX
            nc.vector.tensor_tensor(
                out=o3[:, 0:1176], in0=z1[:, lo : lo + 1176],
                in1=T[:, 0:1].broadcast_to([128, 1176]),
                op=mybir.AluOpType.is_ge,
            )
            nc.scalar.activation(
                out=o3[:, 1176:HFX], in_=z1[:, lo + 1176 : lo + HFX],
                func=mybir.ActivationFunctionType.Sign,
                bias=B1[:, 0:1], scale=A1[:, 0:1],
            )
            nc.sync.dma_start(out_d[3, 1][:, HFX:PIX], o3[:])

    _split_multi_waits(nc)
    return nc


def _prep_inputs(x, weight, gamma, beta):
    """Host-side prep: sign/alpha/gamma folding, padding, 3-term fp8 split."""
    x = np.ascontiguousarray(x, dtype=np.float32)
    weight = np.ascontiguousarray(weight, dtype=np.float32)
    gamma = np.asarray(gamma, np.float32)
    beta = np.asarray(beta, np.float32)

    alpha = np.abs(weight).mean(axis=(1, 2, 3)).astype(np.float32)      # [256]
    S = np.where(gamma >= 0, np.float32(1), np.float32(-1))
    sgn = np.where(weight >= 0, np.float32(1), np.float32(-1)) * S[:, None, None, None]

    # ---- fp8 weight pairs: wq[cin, j, pair, ktile, o] ----
    sgn_t = sgn.transpose(1, 2, 3, 0).reshape(CIN, KH * KW, NCHUNK, 128)
    wq = np.zeros((CIN, NCHUNK, NPAIR, 2, 128), np.float32)
    S1, S2 = 2.0 ** -4, 2.0 ** -6
    for k in range(9):                      # pairs 0-8: (q0, q1) of tap k
        wq[:, :, k, 0] = sgn_t[:, k]
        wq[:, :, k, 1] = sgn_t[:, k] * S1
    for dy in range(3):                     # pairs 9-11: q2 taps (dy,0)+(dy,1)
        wq[:, :, 9 + dy, 0] = sgn_t[:, dy * 3 + 0] * S2
        wq[:, :, 9 + dy, 1] = sgn_t[:, dy * 3 + 1] * S2
    wq[:, :, 12, 0] = sgn_t[:, 2] * S2      # pair 12: q2 taps (0,2)+(1,2)
    wq[:, :, 12, 1] = sgn_t[:, 5] * S2
    wq[:, :, 13, 0] = sgn_t[:, 8] * S2      # pair 13: q2 tap (2,2) + zeros
    wq = np.ascontiguousarray(
        wq.reshape(CIN, NCHUNK * NPAIR * WSLOT)
    ).astype(NP8)

    # pqrb[p, j]: P = a^2/N | Qc = a^2/N^2 | R = a*|g| | beta
    def chunked(v):
        return np.ascontiguousarray(v.reshape(NCHUNK, 128).T)  # [128, 2]
    a2 = alpha * alpha
    pqrb = np.concatenate(
        [chunked(a2 / NTOT), chunked(a2 / NTOT / NTOT),
         chunked(alpha * np.abs(gamma)), chunked(beta)], axis=1
    ).astype(np.float32)                                                # [128, 8]

    # ---- 3-term fp8 split of padded x, with shifted q2 planes ----
    xpad = np.zeros((N_FULL, CIN, HP, WP), np.float32)
    xpad[:, :, 1 : H + 1, 1 : W + 1] = x
    xpad = xpad.reshape(N_FULL, CIN, PADPIX)
    q0 = xpad.astype(NP8)
    r1 = xpad - q0.astype(np.float32)
    q1 = (r1 * 16.0).astype(NP8)
    r2 = r1 - q1.astype(np.float32) * (1.0 / 16.0)
    q2 = (r2 * 64.0).astype(NP8)
    q2p = np.zeros((N_FULL, CIN, PADPIX + 64), NP8)
    q2p[:, :, :PADPIX] = q2
    xq = np.stack(
        [q0, q1, q2, q2p[:, :, 1 : 1 + PADPIX], q2p[:, :, 59 : 59 + PADPIX]],
        axis=2,
    )                                                   # [N, CIN, 5, PADPIX]
    xq = np.ascontiguousarray(xq.reshape(N_FULL, CIN, 5 * PADPIX))

    in_maps = []
    for c in range(N_CORES):
        sl = slice(c * IMGS, (c + 1) * IMGS)
        in_maps.append({
            "xq": np.ascontiguousarray(xq[sl]),
            "wq": wq,
            "pqrb": pqrb,
        })
    return in_maps


def kernel(x, weight, gamma, beta):
    in_maps = _prep_inputs(x, weight, gamma, beta)
    nc = build_bass()
    res = run_bass_kernel_spmd(nc, in_maps, core_ids=list(range(N_CORES)))
    out = np.empty((N_FULL, COUT, H, W), np.float32)
    for c in range(N_CORES):
        o = res.results[c]["out"].astype(np.float32)   # [IMGS,2,128,PIX] fp8
        # DVE compare slices emit {1,0}: img2 fully, img3 up to col HFX+1176
        o[2, 1] = o[2, 1] * 2.0 - 1.0
        o[3, 1, :, 0 : HFX + 1176] = o[3, 1, :, 0 : HFX + 1176] * 2.0 - 1.0
        out[c * IMGS : (c + 1) * IMGS] = o.reshape(IMGS, COUT, H, W)
    return out
